# revision 1
# baseline (speedup 1.0000x reference)
"""Trainium2 Bass kernel for nn_AvgPoolVectorsPerWSI (segment-mean over groups).

Math: x [N=2048, M=512, 7, 7], idx [N] in [0,64)
  out[g, m] = mean over {n: idx[n]==g} and spatial of x[n, m, :, :]  -> [64, 512, 1, 1]

Strategy (no collectives needed):
  - Shard over M: core k handles an m-slice of 64 channels. Each core reads
    its x slice [2048, 64, 49] (25.7 MB) once -> memory-bound (~63-68 us/core
    DMA stream; the NC-pair shares one HBM stack, so ~358 GB/s/core sustained).
  - All compute is fp32-exact. The per-tile work is split across engines so
    both stay well under the ~3.95 us/tile DMA pace:
      * TensorE, m-channels [0, MC): fused segment-sum directly on raw x,
          psum_big[g, (m,j)] += w[n, g]^T @ x[n, (m,j)]
      * VectorE, m-channels [MC, 64): spatial j-reduce to xs[n, m], then a
        tiny fp32 matmul accumulates psum_small[g, m] += w[n, g]^T @ xs[n, m]
    with w the scale-weighted one-hot (scale = 1/(count_g*49)), generated
    ON DEVICE from a 74 KB aux tensor (iota/scale/idx) so the HBM stream is
    just x.
  - Epilogue (pipelined with the last tile's matmul chunks): j-reduce
    psum_big -> out[:, :MC] in three m-chunks, copy psum_small -> out[:, MC:],
    DMA out [64, 64]. Host concatenates the 8 results along m.

Raw Block implementation (not Tile): the walrus matmul/DMA lowerings only
accept ONE attached sync-wait per instruction; standalone wait_ge
instructions sidestep that.

DMA-completion semaphores: tile t uses sem t % BUFS with a cumulative
threshold. A shared counter is only safe because a tile's sem is reused
(t+BUFS) strictly after tile t was consumed (the slot-reuse wait orders the
re-issue); with fewer sems than BUFS, a straggling SDMA engine (engine 15
runs ~20% slow in some periods) could satisfy tile t's count with later
tiles' stripes while t is still in flight.
"""

from contextlib import ExitStack

import numpy as np

import concourse.bass as bass
import concourse.mybir as mybir
from concourse.bass_utils import run_bass_kernel_spmd

N = 2048          # samples
M = 512           # channels
HW = 49           # spatial (7*7)
G = 64            # groups
CORES = 8
ML = M // CORES   # 64 channels per core
F = ML * HW       # 3136 floats per (n, core)
P = 128           # partitions per tile
NT = N // P       # 16 n-tiles
BUFS = 8          # x-tile buffer depth == number of DMA semaphores

MC = 30           # m-channels handled by TensorE (raw fused matmul)
MV = ML - MC      # m-channels handled by VectorE reduce
FC = MC * HW      # 1470 raw columns through the PE
# fp32 matmul chunks must stay within one 2KB PSUM bank -> 512-col chunks
CHUNKS = [(c * 512, min((c + 1) * 512, FC)) for c in range((FC + 511) // 512)]
NCH = len(CHUNKS)
# epilogue sub-reduce m-chunks of psum_big, aligned to the matmul chunks:
# sub-chunk i needs matmul chunks 0..need_i of the last tile (pe_big counts
# one inc per chunk per tile, in chunk order).
SUBRED = []
for _mlo, _mhi in ((0, 10), (10, 20), (20, MC)):
    _need = next(i for i, (lo, hi) in enumerate(CHUNKS) if hi >= _mhi * HW)
    SUBRED.append((_mlo, _mhi, (NT - 1) * NCH + _need + 1))

F32 = mybir.dt.float32


def _build():
    nc = bass.Bass(trn_type="TRN2", target_bir_lowering=False)
    x_ext = nc.declare_dram_parameter("x", [N, F], F32, isOutput=False)
    # aux[:, 0:64] iota row, aux[:, 64:128] scale row, aux[:, 128:144] idx
    aux_ext = nc.declare_dram_parameter("aux", [P, G + G + NT], F32,
                                        isOutput=False)
    out_ext = nc.declare_dram_parameter("out", [G, ML], F32, isOutput=True)

    x_t = x_ext.ap().rearrange("(t p) f -> t p f", p=P)  # [16, 128, 3136]

    with ExitStack() as ctx:
        x_buf = ctx.enter_context(nc.sbuf_tensor([P, BUFS * F], F32))
        xs_buf = ctx.enter_context(nc.sbuf_tensor([P, BUFS * MV], F32))
        aux_sb = ctx.enter_context(nc.sbuf_tensor([P, G + G + NT], F32))
        w_sb = ctx.enter_context(nc.sbuf_tensor([P, NT * G], F32))
        out_sb = ctx.enter_context(nc.sbuf_tensor([G, ML], F32))
        psum_big = ctx.enter_context(nc.psum_tensor([G, FC], F32))
        psum_small = ctx.enter_context(nc.psum_tensor([G, MV], F32))
        dma_x = [
            ctx.enter_context(nc.semaphore(name=f"dma_x{s}"))
            for s in range(BUFS)
        ]
        dma_a = ctx.enter_context(nc.semaphore())   # +16 when aux resident
        dma_o = ctx.enter_context(nc.semaphore())   # +16 when out written
        wg_sem = ctx.enter_context(nc.semaphore())  # +1 when w generated
        red_sem = ctx.enter_context(nc.semaphore())  # +1 per tile j-reduce
        pe_big = ctx.enter_context(nc.semaphore())   # +1 per big matmul chunk
        pe_tile = ctx.enter_context(nc.semaphore())  # +1 per tile (small mm)
        fin_sem = ctx.enter_context(nc.semaphore())  # +4 when out_sb ready
        block = ctx.enter_context(nc.Block())

        def xwait(engine, t):
            engine.wait_ge(dma_x[t % BUFS], 16 * (t // BUFS + 1))

        # ---- DMA program (SP / HWDGE, FIFO) ----
        @block.sync
        def _(sync):
            def xdma(t):
                if t >= BUFS:
                    # slot reuse: the small matmul is ordered after both the
                    # j-reduce and the big matmuls of its tile
                    sync.wait_ge(pe_tile, t - BUFS + 1)
                slot = t % BUFS
                sync.dma_start(
                    out=x_buf[:, slot * F:(slot + 1) * F], in_=x_t[t]
                ).then_inc(dma_x[slot], 16)

            xdma(0)
            sync.dma_start(out=aux_sb[:, :], in_=aux_ext.ap()).then_inc(dma_a, 16)
            for t in range(1, NT):
                xdma(t)
            sync.wait_ge(fin_sem, 4)
            sync.dma_start(out=out_ext.ap(), in_=out_sb[:, :]).then_inc(dma_o, 16)
            sync.wait_ge(dma_o, 16)

        # ---- VectorE: w generation, j-reduction, epilogue ----
        @block.vector
        def _(vector):
            # generate the scale-weighted one-hot from idx:
            #   w[p, t*G+g] = (idx[t*128+p] == g) * scale[g]
            vector.wait_ge(dma_a, 16)
            for t in range(NT):
                wg = vector.scalar_tensor_tensor(
                    out=w_sb[:, t * G:(t + 1) * G],
                    in0=aux_sb[:, 0:G],
                    scalar=aux_sb[:, 2 * G + t:2 * G + t + 1],
                    in1=aux_sb[:, G:2 * G],
                    op0=mybir.AluOpType.is_equal,
                    op1=mybir.AluOpType.mult,
                )
            wg.then_inc(wg_sem, 1)

            for t in range(NT):
                xwait(vector, t)
                if t >= BUFS:
                    # xs slot reuse: wait until tile t-BUFS consumed by PE
                    vector.wait_ge(pe_tile, t - BUFS + 1)
                slot = t % BUFS
                vector.tensor_reduce(
                    out=xs_buf[:, slot * MV:(slot + 1) * MV],
                    in_=x_buf[:, slot * F + FC:(slot + 1) * F].rearrange(
                        "p (m j) -> p m j", j=HW
                    ),
                    axis=mybir.AxisListType.X,
                    op=mybir.AluOpType.add,
                ).then_inc(red_sem, 1)

            # epilogue: j-reduce psum_big in m-chunks as the last tile's
            # matmul chunks complete; copy psum_small
            for mlo, mhi, need in SUBRED:
                vector.wait_ge(pe_big, need)
                vector.tensor_reduce(
                    out=out_sb[:, mlo:mhi],
                    in_=psum_big[:, mlo * HW:mhi * HW].rearrange(
                        "p (m j) -> p m j", j=HW
                    ),
                    axis=mybir.AxisListType.X,
                    op=mybir.AluOpType.add,
                ).then_inc(fin_sem, 1)
            vector.wait_ge(pe_tile, NT)
            vector.tensor_copy(
                out_sb[:, MC:ML], psum_small[:, :]
            ).then_inc(fin_sem, 1)

        # ---- TensorE: segment-sum accumulation (fp32) ----
        @block.tensor
        def _(tensor):
            tensor.wait_ge(wg_sem, 1)
            for t in range(NT):
                xwait(tensor, t)
                slot = t % BUFS
                wt = w_sb[:, t * G:(t + 1) * G]
                for lo, hi in CHUNKS:
                    tensor.matmul(
                        out=psum_big[:, lo:hi],
                        lhsT=wt,
                        rhs=x_buf[:, slot * F + lo:slot * F + hi],
                        start=(t == 0),
                        stop=(t == NT - 1),
                    ).then_inc(pe_big, 1)
                tensor.wait_ge(red_sem, t + 1)
                tensor.matmul(
                    out=psum_small[:, :],
                    lhsT=wt,
                    rhs=xs_buf[:, slot * MV:(slot + 1) * MV],
                    start=(t == 0),
                    stop=(t == NT - 1),
                ).then_inc(pe_tile, 1)

    return nc


def _prepare(x, idx):
    x = np.asarray(x)
    if x.dtype != np.float32:
        x = x.astype(np.float32)
    idx = np.asarray(idx).astype(np.int64)
    counts = np.bincount(idx, minlength=G).astype(np.float64)
    scale = np.where(counts > 0, 1.0 / (counts * HW), 0.0).astype(np.float32)
    aux = np.zeros((P, G + G + NT), np.float32)
    aux[:, 0:G] = np.arange(G, dtype=np.float32)[None, :]
    aux[:, G:2 * G] = scale[None, :]
    aux[:, 2 * G:] = idx.reshape(NT, P).T.astype(np.float32)
    xr = x.reshape(N, M, HW)
    in_maps = []
    for k in range(CORES):
        shard = np.ascontiguousarray(xr[:, k * ML:(k + 1) * ML, :]).reshape(N, F)
        in_maps.append({"x": shard, "aux": aux})
    return in_maps


def run(x, tensor_list_assignmentindices, trace=False):
    in_maps = _prepare(x, tensor_list_assignmentindices)
    nc = _build()
    res = run_bass_kernel_spmd(nc, in_maps, core_ids=list(range(CORES)), trace=trace)
    outs = [np.asarray(r["out"]) for r in res.results]
    out = np.concatenate(outs, axis=1)  # [G, M]
    return out.reshape(G, M, 1, 1).astype(np.float32), res.exec_time_ns


def kernel(**inputs):
    out, _ = run(inputs["x"], inputs["tensor_list_assignmentindices"], trace=False)
    return out



# revision 10
# speedup vs baseline: 1.0155x; 1.0155x over previous
"""Trainium2 Bass kernel for nn_AvgPoolVectorsPerWSI (segment-mean over groups).

Math: x [N=2048, M=512, 7, 7], idx [N] in [0,64)
  out[g, m] = mean over {n: idx[n]==g} and spatial of x[n, m, :, :]  -> [64, 512, 1, 1]

Strategy (no collectives needed):
  - Shard over M: core k handles an m-slice of 64 channels. Each core reads
    its x slice [2048, 64, 49] (25.7 MB) once; the stream runs at the
    ~358 GB/s per-core HBM cap (~72 us), which is the roofline.
  - Per 128-row tile, VectorE spatially j-reduces all 64 channels
    (xs[n, m] = sum_j x[n, m, j]; ~3.6 us/tile vs the 4.5 us DMA pace),
    then TensorE does one tiny fp32 matmul psum[g, m] += w[n, g]^T @ xs[n, m]
    with w the scale-weighted one-hot (scale = 1/(count_g*49)), generated
    ON DEVICE from a 74 KB aux tensor. PSUM is a single [64, 64] bank and
    the epilogue is just two PSUM->SBUF copies + two small output DMAs.
  - Tail control: every tile streams as TWO half DMAs (so VectorE chases
    half-tiles, keeping its lag ~1.8 us instead of 3.8), and the LAST tile
    streams as FOUR shrinking pieces ([22,21,11,10] channels) chased by
    reduce -> matmul -> copy -> out-DMA. The first out DMA (ch [0:43])
    issues on the ACT HWDGE ring mid-stream; only the last small piece's
    chain (+~1.8 us DMA latency) trails the stream.

Raw Block implementation (not Tile): the walrus matmul/DMA lowerings only
accept ONE attached sync-wait per instruction; standalone wait_ge
instructions sidestep that.

DMA-completion semaphores: one sem per in-flight piece (slot-cycled, with
cumulative thresholds). Distinct pieces may NOT share a sem with
intermediate thresholds: SDMA engines progress unevenly (engine 15 runs
~15-20% slow), so a later piece's stripes could satisfy an earlier piece's
count while that piece is still in flight. Per-piece sems + FIFO-per-engine
ordering make each threshold exact.
"""

from contextlib import ExitStack

import numpy as np

import concourse.bass as bass
import concourse.mybir as mybir
from concourse.bass_utils import run_bass_kernel_spmd

N = 2048          # samples
M = 512           # channels
HW = 49           # spatial (7*7)
G = 64            # groups
CORES = 8
ML = M // CORES   # 64 channels per core
F = ML * HW       # 3136 floats per (n, core)
P = 128           # partitions per tile
NT = N // P       # 16 n-tiles
BUFS = 8          # x-tile buffer depth (slots)

# channel-range pieces per tile (each piece = one DMA + one j-reduce)
STEADY = [(0, 32), (32, ML)]            # tiles 0..14
LAST = [(0, 22), (22, 43), (43, 54), (54, ML)]  # tile 15, shrinking tail
# TensorE matmul groups for tile 15: (ch_lo, ch_hi, red_sem threshold)
NRED_STEADY = len(STEADY) * (NT - 1)    # 30 piece-reduces in tiles 0..14
MM15 = [
    (0, 43, NRED_STEADY + 2),           # after tile-15 pieces 0,1 reduced
    (43, ML, NRED_STEADY + 4),          # after pieces 2,3 reduced
]

F32 = mybir.dt.float32


def _build():
    nc = bass.Bass(trn_type="TRN2", target_bir_lowering=False)
    x_ext = nc.declare_dram_parameter("x", [N, F], F32, isOutput=False)
    # aux[:, 0:64] iota row, aux[:, 64:128] scale row, aux[:, 128:144] idx
    aux_ext = nc.declare_dram_parameter("aux", [P, G + G + NT], F32,
                                        isOutput=False)
    out_ext = nc.declare_dram_parameter("out", [G, ML], F32, isOutput=True)

    x_t = x_ext.ap().rearrange("(t p) f -> t p f", p=P)  # [16, 128, 3136]

    def pieces(t):
        return STEADY if t < NT - 1 else LAST

    with ExitStack() as ctx:
        x_buf = ctx.enter_context(nc.sbuf_tensor([P, BUFS * F], F32))
        xs_buf = ctx.enter_context(nc.sbuf_tensor([P, BUFS * ML], F32))
        aux_sb = ctx.enter_context(nc.sbuf_tensor([P, G + G + NT], F32))
        w_sb = ctx.enter_context(nc.sbuf_tensor([P, NT * G], F32))
        out_sb = ctx.enter_context(nc.sbuf_tensor([G, ML], F32))
        psum = ctx.enter_context(nc.psum_tensor([G, ML], F32))
        # per-slot, per-piece DMA-completion sems; tile 15 (slot 7) has 4
        dma_x = [
            [
                ctx.enter_context(nc.semaphore(name=f"dma_x{s}_{p}"))
                for p in range(4 if s == BUFS - 1 else 2)
            ]
            for s in range(BUFS)
        ]
        dma_a = ctx.enter_context(nc.semaphore())   # +16 when aux resident
        dma_o = ctx.enter_context(nc.semaphore())   # +16 per out DMA
        wg_sem = ctx.enter_context(nc.semaphore())  # +1 when w generated
        red_sem = ctx.enter_context(nc.semaphore())  # +1 per piece j-reduce
        pe_sem = ctx.enter_context(nc.semaphore())   # +1 per matmul
        cp_sem = ctx.enter_context(nc.semaphore())   # +1 when out_sb copied
        block = ctx.enter_context(nc.Block())

        def xsem(t, p):
            # piece p of tile t: its sem and cumulative threshold
            # (threshold = 16 x number of increments this sem has seen,
            #  i.e. tiles <= t on this slot that have more than p pieces)
            slot = t % BUFS
            sem = dma_x[slot][p]
            uses = sum(
                1 for tp in range(slot, t + 1, BUFS) if len(pieces(tp)) > p
            )
            return sem, 16 * uses

        # ---- DMA program (SP / HWDGE, FIFO): the x stream ----
        @block.sync
        def _(sync):
            def xdma(t):
                if t >= BUFS:
                    # slot reuse: x_buf slot free once VectorE consumed the
                    # previous tile (its piece-reduces are the only reads)
                    sync.wait_ge(red_sem, len(STEADY) * (t - BUFS + 1))
                slot = t % BUFS
                for p, (clo, chi) in enumerate(pieces(t)):
                    sem, thr = xsem(t, p)
                    lo, hi = clo * HW, chi * HW
                    sync.dma_start(
                        out=x_buf[:, slot * F + lo:slot * F + hi],
                        in_=x_t[t][:, lo:hi],
                    ).then_inc(sem, 16)

            xdma(0)
            sync.dma_start(out=aux_sb[:, :], in_=aux_ext.ap()).then_inc(dma_a, 16)
            for t in range(1, NT):
                xdma(t)
            sync.wait_ge(dma_o, 16)

        # ---- VectorE: w generation, piece j-reductions, final copy B ----
        @block.vector
        def _(vector):
            # w[p, t*G+g] = (idx[t*128+p] == g) * scale[g]
            vector.wait_ge(dma_a, 16)
            for t in range(NT):
                wg = vector.scalar_tensor_tensor(
                    out=w_sb[:, t * G:(t + 1) * G],
                    in0=aux_sb[:, 0:G],
                    scalar=aux_sb[:, 2 * G + t:2 * G + t + 1],
                    in1=aux_sb[:, G:2 * G],
                    op0=mybir.AluOpType.is_equal,
                    op1=mybir.AluOpType.mult,
                )
            wg.then_inc(wg_sem, 1)

            for t in range(NT):
                slot = t % BUFS
                if t >= BUFS:
                    # xs slot reuse: tile t-BUFS consumed by its matmul
                    vector.wait_ge(pe_sem, t - BUFS + 1)
                for p, (clo, chi) in enumerate(pieces(t)):
                    sem, thr = xsem(t, p)
                    vector.wait_ge(sem, thr)
                    vector.tensor_reduce(
                        out=xs_buf[:, slot * ML + clo:slot * ML + chi],
                        in_=x_buf[
                            :, slot * F + clo * HW:slot * F + chi * HW
                        ].rearrange("p (m j) -> p m j", j=HW),
                        axis=mybir.AxisListType.X,
                        op=mybir.AluOpType.add,
                    ).then_inc(red_sem, 1)


        # ---- TensorE: segment-sum accumulation (fp32, tiny matmuls) ----
        @block.tensor
        def _(tensor):
            tensor.wait_ge(wg_sem, 1)
            for t in range(NT - 1):
                slot = t % BUFS
                tensor.wait_ge(red_sem, len(STEADY) * (t + 1))
                tensor.matmul(
                    out=psum[:, :],
                    lhsT=w_sb[:, t * G:(t + 1) * G],
                    rhs=xs_buf[:, slot * ML:(slot + 1) * ML],
                    start=(t == 0),
                    stop=False,
                ).then_inc(pe_sem, 1)
            # tile 15: two column-group matmuls chasing the piece reduces
            t = NT - 1
            slot = t % BUFS
            wt = w_sb[:, t * G:(t + 1) * G]
            for clo, chi, need in MM15:
                tensor.wait_ge(red_sem, need)
                tensor.matmul(
                    out=psum[:, clo:chi],
                    lhsT=wt,
                    rhs=xs_buf[:, slot * ML + clo:slot * ML + chi],
                    start=False,
                    stop=True,
                ).then_inc(pe_sem, 1)

        # ---- ACT (scalar): psum->sbuf copy + out DMA on the 2nd HWDGE ring
        # (copy only after ALL matmuls stopped: PSUM is one bank, and any
        #  concurrent access to a bank the PE still writes is fatal)
        @block.scalar
        def _(scalar):
            scalar.wait_ge(pe_sem, NT + 1)  # 15 tile matmuls + mm A + mm B
            scalar.copy(out_sb[:, :], psum[:, :]).then_inc(cp_sem, 1)
            # engines pipeline dispatch: without this gate the HWDGE DMA can
            # read out_sb before the copy's data has landed
            scalar.wait_ge(cp_sem, 1)
            scalar.dma_start(
                out=out_ext.ap(), in_=out_sb[:, :]
            ).then_inc(dma_o, 16)

    return nc


def _prepare(x, idx):
    x = np.asarray(x)
    if x.dtype != np.float32:
        x = x.astype(np.float32)
    idx = np.asarray(idx).astype(np.int64)
    counts = np.bincount(idx, minlength=G).astype(np.float64)
    scale = np.where(counts > 0, 1.0 / (counts * HW), 0.0).astype(np.float32)
    aux = np.zeros((P, G + G + NT), np.float32)
    aux[:, 0:G] = np.arange(G, dtype=np.float32)[None, :]
    aux[:, G:2 * G] = scale[None, :]
    aux[:, 2 * G:] = idx.reshape(NT, P).T.astype(np.float32)
    xr = x.reshape(N, M, HW)
    in_maps = []
    for k in range(CORES):
        shard = np.ascontiguousarray(xr[:, k * ML:(k + 1) * ML, :]).reshape(N, F)
        in_maps.append({"x": shard, "aux": aux})
    return in_maps


def run(x, tensor_list_assignmentindices, trace=False):
    in_maps = _prepare(x, tensor_list_assignmentindices)
    nc = _build()
    res = run_bass_kernel_spmd(nc, in_maps, core_ids=list(range(CORES)), trace=trace)
    outs = [np.asarray(r["out"]) for r in res.results]
    out = np.concatenate(outs, axis=1)  # [G, M]
    return out.reshape(G, M, 1, 1).astype(np.float32), res.exec_time_ns


def kernel(**inputs):
    out, _ = run(inputs["x"], inputs["tensor_list_assignmentindices"], trace=False)
    return out



# revision 16
# speedup vs baseline: 1.0315x; 1.0158x over previous
"""Trainium2 Bass kernel for nn_AvgPoolVectorsPerWSI (segment-mean over groups).

Math: x [N=2048, M=512, 7, 7], idx [N] in [0,64)
  out[g, m] = mean over {n: idx[n]==g} and spatial of x[n, m, :, :]  -> [64, 512, 1, 1]

Strategy (no collectives needed):
  - Shard over M: core k handles an m-slice of 64 channels. Each core reads
    its x slice [2048, 64, 49] (25.7 MB) once; the stream runs at the
    ~358 GB/s per-core HBM cap (~72 us), which is the roofline.
  - Per 128-row tile, VectorE spatially j-reduces all 64 channels
    (xs[n, m] = sum_j x[n, m, j]; ~3.6 us/tile vs the 4.5 us DMA pace),
    then TensorE does one tiny fp32 matmul psum[g, m] += w[n, g]^T @ xs[n, m]
    with w the scale-weighted one-hot (scale = 1/(count_g*49)), generated
    ON DEVICE from a 74 KB aux tensor. PSUM is a single [64, 64] bank and
    the epilogue is just two PSUM->SBUF copies + two small output DMAs.
  - Tail control: tiles 0..12 stream as single full DMAs (small stripes
    drop the straggler SDMA engine below the HBM cap), tiles 13-14 as two
    halves and tile 15 as FOUR shrinking pieces ([22,21,11,10] channels),
    so VectorE's one-tile reduce lag drains and the last pieces are chased
    by reduce -> matmul -> copy -> out-DMA with only the final small
    piece's chain (+~1.8 us DMA latency) trailing the stream.

Raw Block implementation (not Tile): the walrus matmul/DMA lowerings only
accept ONE attached sync-wait per instruction; standalone wait_ge
instructions sidestep that.

DMA-completion semaphores: one sem per in-flight piece (slot-cycled, with
cumulative thresholds). Distinct pieces may NOT share a sem with
intermediate thresholds: SDMA engines progress unevenly (engine 15 runs
~15-20% slow), so a later piece's stripes could satisfy an earlier piece's
count while that piece is still in flight. Per-piece sems + FIFO-per-engine
ordering make each threshold exact.
"""

from contextlib import ExitStack

import numpy as np

import concourse.bass as bass
import concourse.mybir as mybir
from concourse.bass_utils import run_bass_kernel_spmd

N = 2048          # samples
M = 512           # channels
HW = 49           # spatial (7*7)
G = 64            # groups
CORES = 8
ML = M // CORES   # 64 channels per core
F = ML * HW       # 3136 floats per (n, core)
P = 128           # partitions per tile
NT = N // P       # 16 n-tiles
BUFS = 8          # x-tile buffer depth (slots)

# channel-range pieces per tile (each piece = one DMA + one j-reduce).
# Full-tile DMAs keep the stream at the HBM cap (small stripes slow the
# straggler SDMA engine below it); only the last three tiles are split so
# VectorE's reduce lag drains before the endgame chase.
PIECES = {t: [(0, ML)] for t in range(NT)}
PIECES[NT - 3] = [(0, 32), (32, ML)]
PIECES[NT - 2] = [(0, 32), (32, ML)]
PIECES[NT - 1] = [(0, 22), (22, 43), (43, 54), (54, ML)]


def red_total(t):
    # cumulative piece-reduce count once tile t is fully reduced
    return sum(len(PIECES[k]) for k in range(t + 1))


# TensorE matmul groups for tile 15: (ch_lo, ch_hi, red_sem threshold)
MM15 = [
    (0, 43, red_total(NT - 2) + 2),     # after tile-15 pieces 0,1 reduced
    (43, ML, red_total(NT - 1)),        # after all tile-15 pieces
]

F32 = mybir.dt.float32


def _build():
    nc = bass.Bass(trn_type="TRN2", target_bir_lowering=False)
    x_ext = nc.declare_dram_parameter("x", [N, F], F32, isOutput=False)
    # aux[:, 0:64] iota row, aux[:, 64:128] scale row, aux[:, 128:144] idx
    aux_ext = nc.declare_dram_parameter("aux", [P, G + G + NT], F32,
                                        isOutput=False)
    out_ext = nc.declare_dram_parameter("out", [G, ML], F32, isOutput=True)

    x_t = x_ext.ap().rearrange("(t p) f -> t p f", p=P)  # [16, 128, 3136]

    def pieces(t):
        return PIECES[t]

    with ExitStack() as ctx:
        x_buf = ctx.enter_context(nc.sbuf_tensor([P, BUFS * F], F32))
        xs_buf = ctx.enter_context(nc.sbuf_tensor([P, BUFS * ML], F32))
        aux_sb = ctx.enter_context(nc.sbuf_tensor([P, G + G + NT], F32))
        w_sb = ctx.enter_context(nc.sbuf_tensor([P, NT * G], F32))
        out_sb = ctx.enter_context(nc.sbuf_tensor([G, ML], F32))
        psum = ctx.enter_context(nc.psum_tensor([G, ML], F32))
        # per-slot, per-piece DMA-completion sems (as many as the widest
        # piece list among the tiles cycling through that slot)
        dma_x = [
            [
                ctx.enter_context(nc.semaphore(name=f"dma_x{s}_{p}"))
                for p in range(
                    max(len(PIECES[t]) for t in range(s, NT, BUFS))
                )
            ]
            for s in range(BUFS)
        ]
        dma_a = ctx.enter_context(nc.semaphore())   # +16 when aux resident
        dma_o = ctx.enter_context(nc.semaphore())   # +16 per out DMA
        wg_sem = ctx.enter_context(nc.semaphore())  # +1 when w generated
        red_sem = ctx.enter_context(nc.semaphore())  # +1 per piece j-reduce
        pe_sem = ctx.enter_context(nc.semaphore())   # +1 per matmul
        cp_sem = ctx.enter_context(nc.semaphore())   # +1 when out_sb copied
        block = ctx.enter_context(nc.Block())

        def xsem(t, p):
            # piece p of tile t: its sem and cumulative threshold
            # (threshold = 16 x number of increments this sem has seen,
            #  i.e. tiles <= t on this slot that have more than p pieces)
            slot = t % BUFS
            sem = dma_x[slot][p]
            uses = sum(
                1 for tp in range(slot, t + 1, BUFS) if len(pieces(tp)) > p
            )
            return sem, 16 * uses

        # ---- DMA program (SP / HWDGE, FIFO): the x stream ----
        @block.sync
        def _(sync):
            def xdma(t):
                if t >= BUFS:
                    # slot reuse: x_buf slot free once VectorE consumed the
                    # previous tile (its piece-reduces are the only reads)
                    sync.wait_ge(red_sem, red_total(t - BUFS))
                slot = t % BUFS
                for p, (clo, chi) in enumerate(pieces(t)):
                    sem, thr = xsem(t, p)
                    lo, hi = clo * HW, chi * HW
                    sync.dma_start(
                        out=x_buf[:, slot * F + lo:slot * F + hi],
                        in_=x_t[t][:, lo:hi],
                    ).then_inc(sem, 16)

            xdma(0)
            sync.dma_start(out=aux_sb[:, :], in_=aux_ext.ap()).then_inc(dma_a, 16)
            for t in range(1, NT):
                xdma(t)
            sync.wait_ge(dma_o, 16)

        # ---- VectorE: w generation, piece j-reductions, final copy B ----
        @block.vector
        def _(vector):
            # w[p, t*G+g] = (idx[t*128+p] == g) * scale[g]
            vector.wait_ge(dma_a, 16)
            for t in range(NT):
                wg = vector.scalar_tensor_tensor(
                    out=w_sb[:, t * G:(t + 1) * G],
                    in0=aux_sb[:, 0:G],
                    scalar=aux_sb[:, 2 * G + t:2 * G + t + 1],
                    in1=aux_sb[:, G:2 * G],
                    op0=mybir.AluOpType.is_equal,
                    op1=mybir.AluOpType.mult,
                )
            wg.then_inc(wg_sem, 1)

            for t in range(NT):
                slot = t % BUFS
                if t >= BUFS:
                    # xs slot reuse: tile t-BUFS consumed by its matmul
                    vector.wait_ge(pe_sem, t - BUFS + 1)
                for p, (clo, chi) in enumerate(pieces(t)):
                    sem, thr = xsem(t, p)
                    vector.wait_ge(sem, thr)
                    vector.tensor_reduce(
                        out=xs_buf[:, slot * ML + clo:slot * ML + chi],
                        in_=x_buf[
                            :, slot * F + clo * HW:slot * F + chi * HW
                        ].rearrange("p (m j) -> p m j", j=HW),
                        axis=mybir.AxisListType.X,
                        op=mybir.AluOpType.add,
                    ).then_inc(red_sem, 1)


        # ---- TensorE: segment-sum accumulation (fp32, tiny matmuls) ----
        @block.tensor
        def _(tensor):
            tensor.wait_ge(wg_sem, 1)
            for t in range(NT - 1):
                slot = t % BUFS
                tensor.wait_ge(red_sem, red_total(t))
                tensor.matmul(
                    out=psum[:, :],
                    lhsT=w_sb[:, t * G:(t + 1) * G],
                    rhs=xs_buf[:, slot * ML:(slot + 1) * ML],
                    start=(t == 0),
                    stop=False,
                ).then_inc(pe_sem, 1)
            # tile 15: two column-group matmuls chasing the piece reduces
            t = NT - 1
            slot = t % BUFS
            wt = w_sb[:, t * G:(t + 1) * G]
            for clo, chi, need in MM15:
                tensor.wait_ge(red_sem, need)
                tensor.matmul(
                    out=psum[:, clo:chi],
                    lhsT=wt,
                    rhs=xs_buf[:, slot * ML + clo:slot * ML + chi],
                    start=False,
                    stop=True,
                ).then_inc(pe_sem, 1)

        # ---- ACT (scalar): psum->sbuf copy + out DMA on the 2nd HWDGE ring
        # (copy only after ALL matmuls stopped: PSUM is one bank, and any
        #  concurrent access to a bank the PE still writes is fatal)
        @block.scalar
        def _(scalar):
            scalar.wait_ge(pe_sem, NT + 1)  # 15 tile matmuls + mm A + mm B
            scalar.copy(out_sb[:, :], psum[:, :]).then_inc(cp_sem, 1)
            # engines pipeline dispatch: without this gate the HWDGE DMA can
            # read out_sb before the copy's data has landed
            scalar.wait_ge(cp_sem, 1)
            scalar.dma_start(
                out=out_ext.ap(), in_=out_sb[:, :]
            ).then_inc(dma_o, 16)

    return nc


def _prepare(x, idx):
    x = np.asarray(x)
    if x.dtype != np.float32:
        x = x.astype(np.float32)
    idx = np.asarray(idx).astype(np.int64)
    counts = np.bincount(idx, minlength=G).astype(np.float64)
    scale = np.where(counts > 0, 1.0 / (counts * HW), 0.0).astype(np.float32)
    aux = np.zeros((P, G + G + NT), np.float32)
    aux[:, 0:G] = np.arange(G, dtype=np.float32)[None, :]
    aux[:, G:2 * G] = scale[None, :]
    aux[:, 2 * G:] = idx.reshape(NT, P).T.astype(np.float32)
    xr = x.reshape(N, M, HW)
    in_maps = []
    for k in range(CORES):
        shard = np.ascontiguousarray(xr[:, k * ML:(k + 1) * ML, :]).reshape(N, F)
        in_maps.append({"x": shard, "aux": aux})
    return in_maps


def run(x, tensor_list_assignmentindices, trace=False):
    in_maps = _prepare(x, tensor_list_assignmentindices)
    nc = _build()
    res = run_bass_kernel_spmd(nc, in_maps, core_ids=list(range(CORES)), trace=trace)
    outs = [np.asarray(r["out"]) for r in res.results]
    out = np.concatenate(outs, axis=1)  # [G, M]
    return out.reshape(G, M, 1, 1).astype(np.float32), res.exec_time_ns


def kernel(**inputs):
    out, _ = run(inputs["x"], inputs["tensor_list_assignmentindices"], trace=False)
    return out

